# revision 1
# baseline (speedup 1.0000x reference)
"""Trainium2 Bass kernel for CompressedLinear (VQ codebook linear layer).

Computes: out = x @ W^T + bias, where
  W = (centroids[indices] @ Pi) * row_norms[:, None]

Sharding: out_features (4096) split across 8 cores (512 each); x replicated.
Per-core device pipeline:
  1. Gather yts[j,o] = centroids[idxT[j,o]] via fused custom-DVE ops (2
     codebook entries per instruction, 8 instructions per tile).
  2. W_u^T[i,o] = sum_j Pi[j,i] * yts[j,o] on the PE (bf16, f32 psum).
  3. outT[o,t] = sum_i W_u^T[i,o] * xT[i,t]; then out = rn*acc + bias on DVE.
Host feeds x pre-transposed/bf16-cast (layout prep), Pi in column-stripe
layout, indices transposed; host reassembles the 8 outT shards.
"""

import numpy as np

# Problem geometry (hardcoded per contract)
OUT, IN = 4096, 4096
B, S = 4, 2048
T = B * S          # 8192 tokens
NCORES = 8
P = 128            # partitions

_DVE_OPS = None
_NC_CACHE = {}


def _register_dve_ops():
    """Register the two fused VQ-gather ops in dve_ops.OPS (idempotent)."""
    global _DVE_OPS
    if _DVE_OPS is not None:
        return _DVE_OPS
    import concourse.dve_ops as dvo
    from concourse.dve_spec import Spec, Src0, Src1, C0, C1, C2, One, eq, lower
    from concourse.dve_uop import DveOpSpec

    existing = {op.name: op for op in dvo.OPS}
    if "VQ_PAIR" in existing:
        _DVE_OPS = {k: existing[k] for k in ("VQ_PAIR", "VQ_ACC2")}
        return _DVE_OPS

    ver = "v3"  # TRN2

    def mk(name, spec, rd1):
        opcode = dvo._CUSTOM_DVE_ROW_BASE + len(dvo.OPS)
        dvo._SUB_OPCODE_FOR_NAME[name] = opcode
        s = DveOpSpec(name=name, opcode=opcode, uops=lower(spec, ver=ver), rd1_en=rd1)
        op = dvo.DveOp(name, spec, subdim=False, uops_sha={ver: s.sha(ver)})
        dvo.OPS.append(op)
        dvo.CUSTOM_DVE_SPECS[name] = spec
        return op

    # out = (idx==imm2)*s0 + (idx==imm2+1)*s1
    pair = mk(
        "VQ_PAIR",
        Spec(
            body=eq(Src0, C2) * C0 + eq(Src0, C2 + One) * C1,
            reference=lambda in0, in1, s0, s1, imm2: (
                (in0 == imm2) * s0 + (in0 == imm2 + 1) * s1
            ).astype(np.float32),
        ),
        False,
    )
    # out = acc + (idx==imm2)*s0 + (idx==imm2+1)*s1
    acc = mk(
        "VQ_ACC2",
        Spec(
            body=Src1 + eq(Src0, C2) * C0 + eq(Src0, C2 + One) * C1,
            reference=lambda in0, in1, s0, s1, imm2: (
                in1 + (in0 == imm2) * s0 + (in0 == imm2 + 1) * s1
            ).astype(np.float32),
        ),
        True,
    )
    _DVE_OPS = {"VQ_PAIR": pair, "VQ_ACC2": acc}
    return _DVE_OPS


def build_nc(cvals, in_=IN, t=T, osh=OUT // NCORES, tch=512, igrp=4):
    """Build the SPMD Bass program. cvals: 16 python floats (codebook)."""
    import concourse.bacc as bacc
    import concourse.mybir as mybir
    from concourse.tile import TileContext

    f32 = mybir.dt.float32
    bf16 = mybir.dt.bfloat16

    nj = in_ // P          # j blocks (rows of Pi / x input dim)
    ni = in_ // P          # i blocks (cols of Pi / contraction of main mm)
    nob = osh // P         # output feature blocks per core
    nt = t // tch          # token chunks
    ngrp = ni // igrp      # i-groups for the W^T stage

    nc = bacc.Bacc()
    xT_d = nc.dram_tensor("xT", [in_, t], bf16, kind="ExternalInput")
    piR_d = nc.dram_tensor("PiR", [ni, in_, P], bf16, kind="ExternalInput")
    idxT_d = nc.dram_tensor("idxT", [in_, osh], bf16, kind="ExternalInput")
    rn_d = nc.dram_tensor("rn", [osh], f32, kind="ExternalInput")
    bias_d = nc.dram_tensor("bias", [osh], f32, kind="ExternalInput")
    outT_d = nc.dram_tensor("outT", [osh, t], f32, kind="ExternalOutput")

    with TileContext(nc) as tc:
        with (
            tc.tile_pool(name="constp", bufs=1) as constp,
            tc.tile_pool(name="idxp", bufs=3) as idxp,
            tc.tile_pool(name="ytsp", bufs=1) as ytsp,
            tc.tile_pool(name="pip", bufs=igrp + 2) as pip,
            tc.tile_pool(name="wtp", bufs=1) as wtp,
            tc.tile_pool(name="xtp", bufs=2) as xtp,
            tc.tile_pool(name="outp", bufs=4) as outp,
            tc.tile_pool(name="wpsum", bufs=1, space="PSUM") as wpsum,
            tc.tile_pool(name="mpsum", bufs=2, space="PSUM") as mpsum,
        ):
            rn_sb = constp.tile([P, nob], f32, name="rn_sb")
            nc.sync.dma_start(rn_sb[:], rn_d.rearrange("(b p) -> p b", p=P))
            bias_sb = constp.tile([P, nob], f32, name="bias_sb")
            nc.sync.dma_start(bias_sb[:], bias_d.rearrange("(b p) -> p b", p=P))

            # ---- Stage 1: codebook gather: yts[j][p, o] = centroids[idxT] --
            # Telescoping form: c[idx] = c0 + sum_{k=1..15} (idx>=k)*(ck-ck-1)
            # Stock ops only; tiles split between DVE and GPSIMD engines.
            dk = [float(cvals[k] - cvals[k - 1]) for k in range(1, 16)]
            yts = []
            for j in range(nj):
                eng = nc.vector
                enm = "v"
                idx_t = idxp.tile([P, osh], bf16, name="idx_t", tag=f"idx{enm}")
                nc.sync.dma_start(idx_t[:], idxT_d[j * P:(j + 1) * P, :])
                acc = idxp.tile([P, osh], f32, name="acc", tag=f"acc{enm}",
                                bufs=2)
                eng.tensor_scalar(acc[:], idx_t[:], 1.0, dk[0],
                                  mybir.AluOpType.is_ge, mybir.AluOpType.mult)
                tmp = idxp.tile([P, osh], f32, name="tmp", tag=f"tmp{enm}",
                                bufs=2)
                for k in range(2, 16):
                    eng.tensor_scalar(tmp[:], idx_t[:], float(k), dk[k - 1],
                                      mybir.AluOpType.is_ge,
                                      mybir.AluOpType.mult)
                    eng.tensor_tensor(acc[:], acc[:], tmp[:],
                                      mybir.AluOpType.add)
                y_t = ytsp.tile([P, osh], bf16, name="y_t", tag=f"yts{j}")
                eng.tensor_scalar(y_t[:], acc[:], float(cvals[0]), None,
                                  mybir.AluOpType.add)
                yts.append(y_t)

            # ---- Stage 2: wt[i_blk][p_i, o] = sum_j Pi[j, i] * yts[j, o] ---
            wts = []
            for ig in range(ngrp):
                pi_ts = []
                for k in range(igrp):
                    i_blk = ig * igrp + k
                    pi_t = pip.tile([P, nj, P], bf16, name="pi_t", tag="pi")
                    nc.sync.dma_start(
                        pi_t[:], piR_d[i_blk].rearrange("(a p) i -> p a i", p=P)
                    )
                    pi_ts.append(pi_t)
                ps = [
                    wpsum.tile([P, osh], f32, name="wps", tag=f"wps{k}")
                    for k in range(igrp)
                ]
                for j in range(nj):
                    for k in range(igrp):
                        nc.tensor.matmul(
                            ps[k][:], pi_ts[k][:, j, :], yts[j][:],
                            start=(j == 0), stop=(j == nj - 1),
                        )
                for k in range(igrp):
                    i_blk = ig * igrp + k
                    wt_t = wtp.tile([P, osh], bf16, name="wt_t", tag=f"wt{i_blk}")
                    nc.scalar.copy(wt_t[:], ps[k][:])
                    wts.append(wt_t)

            # ---- Stage 3: outT[o, t] = rn[o] * sum_i wt[i,o]*xT[i,t] + b[o]
            for tb in range(nt):
                xt_t = xtp.tile([P, ni, tch], bf16, name="xt_t", tag="xt")
                nc.sync.dma_start(
                    xt_t[:],
                    xT_d[:, tb * tch:(tb + 1) * tch].rearrange(
                        "(a p) t -> p a t", p=P
                    ),
                )
                for ob in range(nob):
                    mp = mpsum.tile([P, tch], f32, name="mp", tag="mp")
                    for i_blk in range(ni):
                        nc.tensor.matmul(
                            mp[:], wts[i_blk][:, ob * P:(ob + 1) * P],
                            xt_t[:, i_blk, :],
                            start=(i_blk == 0), stop=(i_blk == ni - 1),
                        )
                    o_t = outp.tile([P, tch], f32, name="o_t", tag="out")
                    nc.vector.tensor_scalar(
                        o_t[:], mp[:], rn_sb[:, ob:ob + 1], bias_sb[:, ob:ob + 1],
                        mybir.AluOpType.mult, mybir.AluOpType.add,
                    )
                    nc.scalar.dma_start(
                        outT_d[ob * P:(ob + 1) * P, tb * tch:(tb + 1) * tch],
                        o_t[:],
                    )
    nc.compile()
    return nc


def _prep_inputs(x, indices, Pi, row_norms, bias):
    """Host-side layout prep + sharding. Returns list of per-core in_maps."""
    import ml_dtypes

    bf16 = ml_dtypes.bfloat16
    x2 = np.ascontiguousarray(
        np.asarray(x, np.float32).reshape(T, IN).T
    ).astype(bf16)  # (IN, T)
    ni = IN // P
    piR = np.ascontiguousarray(
        np.asarray(Pi, np.float32).astype(bf16).reshape(IN, ni, P).transpose(1, 0, 2)
    )  # (ni, IN_j, P_i)
    idxT = np.ascontiguousarray(np.asarray(indices).T).astype(bf16)  # (IN, OUT)
    rn = np.asarray(row_norms, np.float32)
    bs = np.asarray(bias, np.float32)

    osh = OUT // NCORES
    in_maps = []
    for c in range(NCORES):
        sl = slice(c * osh, (c + 1) * osh)
        in_maps.append({
            "xT": x2,
            "PiR": piR,
            "idxT": np.ascontiguousarray(idxT[:, sl]),
            "rn": np.ascontiguousarray(rn[sl]),
            "bias": np.ascontiguousarray(bs[sl]),
        })
    return in_maps


def _get_nc(centroids):
    key = np.asarray(centroids, np.float32).tobytes()
    nc = _NC_CACHE.get(key)
    if nc is None:
        cvals = [float(v) for v in np.asarray(centroids, np.float32)]
        assert len(cvals) == 16
        nc = build_nc(cvals)
        _NC_CACHE.clear()
        _NC_CACHE[key] = nc
    return nc


def kernel(x, indices, centroids, Pi, row_norms, bias):
    from concourse.bass_utils import run_bass_kernel_spmd

    nc = _get_nc(centroids)
    in_maps = _prep_inputs(x, indices, Pi, row_norms, bias)
    res = run_bass_kernel_spmd(nc, in_maps, list(range(NCORES)))
    shards = [np.asarray(res.results[c]["outT"]) for c in range(NCORES)]
    full = np.concatenate(shards, axis=0)           # (OUT, T)
    out = np.ascontiguousarray(full.T).reshape(B, S, OUT)
    return out.astype(np.float32)

